# revision 41
# baseline (speedup 1.0000x reference)
"""Trainium2 Bass kernel for nn_Head_75118978007668.

Computes, for x:[B,S,D], concept_map(cm):[D,D,D] (B=4, S=2048, D=128):
    s[b,t] = sum_{j<t} lam^(t-j) x[b,j]          (lam = 1/1.2 decayed prefix sum)
    out[b,t,f] = sum_{d,e} x[b,t,d] * s[b,t,e] * cm[f,d,e]

Sharding: 8 cores, each owns 1024 contiguous positions of one batch row
(4 rows x 2 halves).  The scan carry across the half-split is recovered
exactly (to fp32) from a 128-position halo, since lam^128 ~ 7e-11 is far
below fp32 resolution.  For the same reason the cross-tile carry
recurrence c_t = lam^128 c_{t-1} + v_{t-1} truncates to c_t = v_{t-1}:
no serial carry chain is needed at all.

Per-core dataflow (positions tiled 8 x 128):
  - carries: 3 small PE matmuls (halo + per-tile decayed sums v)
  - s tiles: triangular matmul  s = L @ x_tile + pow (x) carry  (fp16 in,
    f32 psum), interleaved into the main loop two tiles ahead
  - main:    Y[p, (e,f)] = xT_tile.T @ W2   (PE, fp16, psum groups of 8 e)
    The e-contraction acc[p,f] += s[p,e] * Y[p,(e,f)] is split per 8-e group
    by ROUTE across engines (running concurrently with PE):
      'T': one fused DVE tensor_tensor  y_sb = Y_psum * s  (s broadcast
           along f via a stride-0 AP dim), fp16 out; PE then accumulates
           sum_e I @ y_sb_e into a PSUM bank (identity fp16 matmuls)
      'S': ACT per-e scaled copies (activation Copy, per-partition scale
           s[:,e]) feeding the same identity-matmul accumulation
    Identity-matmul folds are pipelined ACROSS tile boundaries (pending
    queue carries over) so PE never drains at a tile edge.
  where W2[d, e*128+f] = cm[f, d, e]  (host-transposed, fp16).
"""

import numpy as np

import concourse.bass as bass
import concourse.tile as tile
from concourse import bacc, mybir
from concourse.bass import ds, ts
from concourse.bass_utils import run_bass_kernel_spmd

B, S, D = 4, 2048, 128
NCORES = 8
CHUNK = S // 2          # positions per core (1024)
NT = CHUNK // 128       # position tiles per core (8)
P = 128
HALO = 128
F32 = mybir.dt.float32
F16 = mybir.dt.float16
F8 = mybir.dt.float8e4

# fp8e4 y + DoubleRow identity folds: one K=256 matmul folds a whole
# 8-e group at 0.5 cy/col (vs 2 fp16 matmuls at 1 cy/col).  Measured
# rel err 3.3e-2 on HW (> 2e-2 gate): fp8 y quantization is too coarse.
FOLD_FP8 = False
Y_DT = F8 if FOLD_FP8 else F16

# match the reference's fp32 constant 1.2 exactly
LAM = 1.0 / np.float64(np.float32(1.2))

MAIN_MM_DTYPE = F16     # fp16 halves W2 DMA; 1 cy/row on PE

NGRP = 16               # e-groups of 8 per tile
# y is stored TRANSPOSED per group: [p, f(128), e(8)], fp16, so the
# s-scale multiply can run on DVE in its 4x fast mode (all-SBUF fp16,
# packed last dim).  Per-group routes:
#   'T': fused DVE tensor_tensor psum-Y * s -> y (1x, PSUM-read bound)
#   'A': ACT plain evac copy psum-Y -> y_raw, then DVE in-place 4x
#        fast multiply y *= s-broadcast
# GPSIMD cannot touch PSUM; it PRE-FOLDS e-pairs in SBUF (8 -> 4) for
# PREFOLD_GP groups (DVE 4x handles PREFOLD_DVE), halving those groups'
# PE identity-fold matmuls.
ROUTE = "TAATAATAATAATAAT"
assert len(ROUTE) == NGRP
PREFOLD_DVE = frozenset((2, 6, 11, 14))
PREFOLD_GP = frozenset((1, 3, 4, 7, 8, 9, 12, 13))
PREFOLD = PREFOLD_DVE | PREFOLD_GP
N_DMM = (NGRP if FOLD_FP8 else
         sum(1 if g in PREFOLD else 2 for g in range(NGRP)))

_CACHE = {}
LAST_RESULTS = None


def _host_constants():
    k = np.arange(P, dtype=np.float64)
    i = k
    # LT[i, k] = L[k, i] = lam^(k-i) for i < k   (lhsT of the triangular scan)
    LT = np.where(i[:, None] < k[None, :], LAM ** (k[None, :] - i[:, None]), 0.0)
    powv = (LAM ** k)[None, :]                      # [1, 128]
    vw = (LAM ** (P - i))[:, None]                  # [128, 1]
    f16 = np.float16
    eye = np.eye(P, dtype=np.float32)
    if FOLD_FP8:
        # DoubleRow stationary: [k, r, m] = I[k, m] for both k-tiles r
        mask = np.ascontiguousarray(
            np.stack([eye, eye], axis=1).astype(mybir.dt.np(F8)))
    else:
        mask = eye.astype(f16)
    return {
        "lt": LT.astype(f16),
        "powv": powv.astype(f16),
        "vw": vw.astype(f16),
        "mask": mask,
    }


def _build_nc():
    nc = bacc.Bacc("TRN2", target_bir_lowering=False, debug=False,
                   num_devices=NCORES)
    x_d = nc.declare_dram_parameter("x", [P, NT, P], F16, isOutput=False)        # [i, t, e]
    xt_d = nc.declare_dram_parameter("xt", [P, CHUNK], MAIN_MM_DTYPE, isOutput=False)  # [d, p]
    halo_d = nc.declare_dram_parameter("halo", [P, P], F16, isOutput=False)      # [i, e]
    w2_d = nc.declare_dram_parameter("w2", [P, P * P], MAIN_MM_DTYPE, isOutput=False)  # [d, (e,f)]
    lt_d = nc.declare_dram_parameter("lt", [P, P], F16, isOutput=False)
    pow_d = nc.declare_dram_parameter("powv", [1, P], F16, isOutput=False)
    vw_d = nc.declare_dram_parameter("vw", [P, 1], F16, isOutput=False)
    if FOLD_FP8:
        mask_d = nc.declare_dram_parameter("mask", [P, 2, P], F8, isOutput=False)
    else:
        mask_d = nc.declare_dram_parameter("mask", [P, P], F16, isOutput=False)
    out_d = nc.declare_dram_parameter("out", [P, NT, P], F32, isOutput=True)  # [p, t, f]

    mult = mybir.AluOpType.mult
    add = mybir.AluOpType.add

    with tile.TileContext(nc) as tc:
        with tc.tile_pool(name="consts", bufs=1) as consts:
            w2_sb = [consts.tile([P, 2048], MAIN_MM_DTYPE, name=f"w2_sb{i}")
                     for i in range(8)]
            xt_sb = consts.tile([P, CHUNK], MAIN_MM_DTYPE)
            x_sb = consts.tile([P, NT, P], F16)
            halo_sb = consts.tile([P, P], F16)
            lt_sb = consts.tile([P, P], F16)
            pow_sb = consts.tile([1, P], F16)
            vw_sb = consts.tile([P, 1], F16)
            if FOLD_FP8:
                mask_sb = consts.tile([P, 2, P], F8)
            else:
                mask_sb = consts.tile([P, P], F16)
            c_all = consts.tile([1, NT * P], F16)    # [1, (t,e)] carries
            s_sb = consts.tile([P, NT, P], F16)      # [p, t, e]
            acc = consts.tile([P, NT, P], F32)       # [p, t, f]
            y_sb = consts.tile([P, 2 * NGRP, P, 8], F16)       # [p, gi, f, e]
            y2_sb = consts.tile([P, 2 * NGRP, P, 4], F16)      # prefold out

            # critical-path-first DMA order: tiny weights, then halo/x
            # (carries + s-phase inputs), then xt; W2 streams on the scalar
            # DGE queue in consumption order so it never blocks these.
            nc.sync.dma_start(out=vw_sb[:, :], in_=vw_d[:, :])
            nc.sync.dma_start(out=halo_sb[:, :], in_=halo_d[:, :])
            nc.sync.dma_start(out=lt_sb[:, :], in_=lt_d[:, :])
            nc.sync.dma_start(out=pow_sb[:, :], in_=pow_d[:, :])
            if FOLD_FP8:
                nc.sync.dma_start(out=mask_sb[:, :, :], in_=mask_d[:, :, :])
            else:
                nc.sync.dma_start(out=mask_sb[:, :], in_=mask_d[:, :])
            nc.sync.dma_start(out=x_sb[:, 0:4, :], in_=x_d[:, 0:4, :])
            nc.sync.dma_start(out=x_sb[:, 4:8, :], in_=x_d[:, 4:8, :])
            nc.sync.dma_start(out=xt_sb[:, :], in_=xt_d[:, :])
            for c in range(8):
                nc.scalar.dma_start(
                    out=w2_sb[c][:, :],
                    in_=w2_d[:, ds(2048 * c, 2048)])

            # ---- carries: c_t = s(tile_start t); c_t = v_{t-1} exactly ----
            with tc.tile_pool(name="psum_c", bufs=1, space="PSUM") as psum_c:
                c0_ps = psum_c.tile([1, P], F32)
                nc.tensor.matmul(c0_ps[:, :], lhsT=vw_sb[:, :],
                                 rhs=halo_sb[:, :], start=True, stop=True)
                vps_a = psum_c.tile([1, 4 * P], F32, tag="vps_a")
                vps_b = psum_c.tile([1, 4 * P], F32, tag="vps_b")
                nc.tensor.matmul(vps_a[:, :], lhsT=vw_sb[:, :],
                                 rhs=x_sb[:, 0:4, :], start=True, stop=True)
                nc.tensor.matmul(vps_b[:, :], lhsT=vw_sb[:, :],
                                 rhs=x_sb[:, 4:8, :], start=True, stop=True)
                nc.vector.tensor_copy(c_all[:, 0:P], c0_ps[:, :])
                nc.vector.tensor_copy(c_all[:, P:5 * P], vps_a[:, :])
                nc.vector.tensor_copy(c_all[:, 5 * P:8 * P], vps_b[:, 0:3 * P])

            # ---- main loop (s tiles interleaved two ahead) ----
            # psum_y takes 6 banks; one bank holds the two s slots and one
            # the two dacc slots ([P, 2, P] f32 = 1KB/partition each, padded
            # to a bank).  Slot WAR/RAW deps are tracked at the AP level by
            # the tile framework; within each bank, accumulation groups are
            # serialized in program order.
            with tc.tile_pool(name="psum_s", bufs=1, space="PSUM") as psum_s, \
                 tc.tile_pool(name="psum_d", bufs=1, space="PSUM") as psum_d, \
                 tc.tile_pool(name="psum_y", bufs=3, space="PSUM") as psum_y:
                sslots = psum_s.tile([P, 2, P], F32)
                dslots = psum_d.tile([P, 2, P], F32)

                def emit_s(t):
                    sp = sslots[:, t % 2, :]
                    nc.tensor.matmul(sp, lhsT=lt_sb[:, :],
                                     rhs=x_sb[:, t, :], start=True, stop=False)
                    nc.tensor.matmul(sp, lhsT=pow_sb[:, :],
                                     rhs=c_all[:, ts(t, P)], start=False, stop=True)
                    nc.scalar.copy(s_sb[:, t, :], sp)

                emit_s(0)
                emit_s(1)

                DELAYS = {"T": 3, "A": 5}   # groups between evac and id-MMs
                fold_state = {}             # t -> [d2_ap, n_emitted]
                pending = []                # (gi, due_gabs, t), FIFO

                def emit_id_mms(gq, tq):
                    # Folds e-slices (inner dim of the transposed y) into the
                    # 128 accumulator columns: the stride-0 inner out dim
                    # revisits each PSUM element per e, and has_written
                    # semantics turn the revisits into accumulation.
                    st = fold_state[tq]
                    d2 = st[0]
                    if (gq % NGRP) in PREFOLD:
                        # iterate e-outer so same-address PSUM revisits are
                        # 128 columns apart (back-to-back revisits break the
                        # accumulate pipeline)
                        out4 = bass.AP(d2.tensor, d2.offset,
                                       [d2.ap[0], [0, 4], [1, P]])
                        y2v = y2_sb[:, gq, :, :]
                        rhs4 = bass.AP(y2v.tensor, y2v.offset,
                                       [y2v.ap[0], [1, 4], [4, P]])
                        nc.tensor.matmul(
                            out4, lhsT=mask_sb[:, :], rhs=rhs4,
                            start=(st[2] == 0),
                            stop=(st[2] == N_DMM - 1))
                        st[2] += 1
                    else:
                        y3 = y_sb[:, gq, :, :]
                        for h in range(2):
                            out8 = bass.AP(d2.tensor, d2.offset + 64 * h,
                                           [d2.ap[0], [0, 8], [1, 64]])
                            rhsh = bass.AP(y3.tensor, y3.offset + 64 * h * 8,
                                           [y3.ap[0], [1, 8], [8, 64]])
                            nc.tensor.matmul(
                                out8, lhsT=mask_sb[:, :], rhs=rhsh,
                                start=(st[2] == 0),
                                stop=(st[2] == N_DMM - 1))
                            st[2] += 1

                def emit_merge(tq):
                    st = fold_state.pop(tq)
                    assert st[2] == N_DMM
                    nc.vector.tensor_copy(acc[:, tq, :], st[0])
                    nc.sync.dma_start(out=out_d[:, tq, :], in_=acc[:, tq, :])

                for t in range(NT):
                    xt_t = xt_sb[:, ts(t, P)]
                    d2 = dslots[:, t % 2, :]
                    fold_state[t] = [d2, None, 0]

                    for g in range(NGRP):
                        gabs = t * NGRP + g
                        # strict-FIFO drain keeps dacc accumulation groups
                        # serialized in the shared bank
                        while pending and gabs >= pending[0][1]:
                            gq, _, tq = pending.pop(0)
                            emit_id_mms(gq, tq)
                        if g == 7 and t >= 1:
                            emit_merge(t - 1)
                        if g == 8 and t + 2 < NT:
                            emit_s(t + 2)
                        yp = psum_y.tile([P, 8, P], F32)
                        for h in range(2):
                            c = 2 * g + h
                            nc.tensor.matmul(
                                yp[:, ds(4 * h, 4), :], lhsT=xt_t,
                                rhs=w2_sb[c // 4][:, ds(512 * (c % 4), 512)],
                                start=True, stop=True)
                        r = ROUTE[g]
                        gi = (t % 2) * NGRP + g
                        yv = y_sb[:, gi, :, :]          # [p, f, e]
                        # transposed write AP: iterate (e, f) like yp
                        yvT = bass.AP(yv.tensor, yv.offset,
                                      [yv.ap[0], [1, 8], [8, P]])
                        s4 = s_sb[:, t, ds(8 * g, 8)]

                        if r == "T":
                            # fused: y[f,e] = Y[e,f] * s[e]  (1x, PSUM read)
                            in1 = bass.AP(s4.tensor, s4.offset,
                                          [s4.ap[0], [1, 8], [0, P]])
                            nc.vector.tensor_tensor(
                                out=yvT, in0=yp[:, :, :], in1=in1, op=mult)
                        else:
                            # 'A': ACT plain evac (transposed), then DVE 4x
                            # in-place multiply (all-SBUF fp16 packed)
                            nc.scalar.copy(yvT, yp[:, :, :])
                            in1 = bass.AP(s4.tensor, s4.offset,
                                          [s4.ap[0], [0, P], [1, 8]])
                            nc.vector.tensor_tensor(
                                out=yv, in0=yv, in1=in1, op=mult)
                        delay = DELAYS[r]
                        if g in PREFOLD:
                            eng = (nc.gpsimd if g in PREFOLD_GP else nc.vector)
                            eng.tensor_tensor(
                                out=y2_sb[:, gi, :, :],
                                in0=y_sb[:, gi, :, ds(0, 4)],
                                in1=y_sb[:, gi, :, ds(4, 4)],
                                op=add)
                            delay += 2
                        pending.append((gi, gabs + delay, t))
                while pending:
                    gq, _, tq = pending.pop(0)
                    emit_id_mms(gq, tq)
                emit_merge(NT - 1)
    nc.finalize()
    return nc


def _get_nc():
    if "nc" not in _CACHE:
        _CACHE["nc"] = _build_nc()
    return _CACHE["nc"]


def kernel(x, concept_map, _trace=False):
    global LAST_RESULTS
    x = np.asarray(x, dtype=np.float32)
    cm = np.asarray(concept_map, dtype=np.float32)
    assert x.shape == (B, S, D) and cm.shape == (D, D, D)

    consts = _host_constants()
    # W2[d, e*128+f] = cm[f, d, e]
    w2 = np.ascontiguousarray(
        np.transpose(cm, (1, 2, 0)).reshape(D, D * D).astype(np.float16))

    in_maps = []
    for core in range(NCORES):
        b, half = divmod(core, 2)
        lo = half * CHUNK
        xc = x[b, lo:lo + CHUNK]                          # [1024, 128]
        # [i, t, e] interleaved layout (partition = within-tile position)
        x_il = np.ascontiguousarray(
            xc.reshape(NT, P, D).transpose(1, 0, 2).astype(np.float16))
        xt = np.ascontiguousarray(xc.T.astype(np.float16))  # [d, p]
        if half == 0:
            halo = np.zeros((P, D), dtype=np.float16)
        else:
            halo = np.ascontiguousarray(
                x[b, lo - HALO:lo].astype(np.float16))    # [128, 128]
        in_maps.append({
            "x": x_il, "xt": xt, "halo": halo, "w2": w2, **consts,
        })

    nc = _get_nc()
    res = run_bass_kernel_spmd(nc, in_maps, list(range(NCORES)), trace=_trace)
    LAST_RESULTS = res

    out = np.empty((B, S, D), dtype=np.float32)
    for core in range(NCORES):
        b, half = divmod(core, 2)
        o = res.results[core]["out"]                      # [p, t, f]
        out[b, half * CHUNK:(half + 1) * CHUNK] = (
            o.transpose(1, 0, 2).reshape(CHUNK, D))
    return out


# revision 42
# speedup vs baseline: 2.5886x; 2.5886x over previous
"""Trainium2 Bass kernel for nn_Head_75118978007668.

Computes, for x:[B,S,D], concept_map(cm):[D,D,D] (B=4, S=2048, D=128):
    s[b,t] = sum_{j<t} lam^(t-j) x[b,j]          (lam = 1/1.2 decayed prefix sum)
    out[b,t,f] = sum_{d,e} x[b,t,d] * s[b,t,e] * cm[f,d,e]

Sharding: 8 cores, each owns 1024 contiguous positions of one batch row
(4 rows x 2 halves).  The scan carry across the half-split is recovered
exactly (to fp32) from a 128-position halo, since lam^128 ~ 7e-11 is far
below fp32 resolution.  For the same reason the cross-tile carry
recurrence c_t = lam^128 c_{t-1} + v_{t-1} truncates to c_t = v_{t-1}:
no serial carry chain is needed.

Per-core dataflow (positions tiled 8 x 128):
  - carries: 3 small PE matmuls (halo + per-tile decayed sums v)
  - s tiles: triangular matmul  s = L @ x_tile + pow (x) carry  (fp16 in,
    f32 psum), interleaved into the main loop two tiles ahead
  - main:    Y[p, (e,f)] = xT_tile.T @ W2   (PE, fp16, psum groups of 8 e)
    The e-contraction acc[p,f] += s[p,e] * Y[p,(e,f)] is split per 8-e group
    by ROUTE across engines (running concurrently with PE):
      'T': one fused DVE tensor_tensor  y_sb = Y_psum * s  (s broadcast
           along f via a stride-0 AP dim), fp16 out
      'S': ACT per-e scaled copies (activation Copy, per-partition scale
           s[:,e])
    then PE accumulates sum_e I @ y_sb_e into a PSUM bank (identity fp16
    matmuls).  For PREFOLD_GP groups, the idle GPSIMD engine first adds
    e-slice pairs in SBUF (8 -> 4), halving those groups' PE fold work.
    Folds are drained strict-FIFO and pipelined across tile boundaries.
  where W2[d, e*128+f] = cm[f, d, e]  (host-transposed, fp16).
"""

import numpy as np

import concourse.bass as bass
import concourse.tile as tile
from concourse import bacc, mybir
from concourse.bass import ds, ts
from concourse.bass_utils import run_bass_kernel_spmd

B, S, D = 4, 2048, 128
NCORES = 8
CHUNK = S // 2          # positions per core (1024)
NT = CHUNK // 128       # position tiles per core (8)
P = 128
HALO = 128
F32 = mybir.dt.float32
F16 = mybir.dt.float16

# match the reference's fp32 constant 1.2 exactly
LAM = 1.0 / np.float64(np.float32(1.2))

NGRP = 16               # e-groups of 8 per tile
ROUTE = "TSTTTSTTTSTTSTTT"
assert len(ROUTE) == NGRP
PREFOLD_GP = frozenset((1, 3, 6, 9, 11, 14))
N_DMM = sum(1 if g in PREFOLD_GP else 2 for g in range(NGRP))

_CACHE = {}
LAST_RESULTS = None


def _host_constants():
    k = np.arange(P, dtype=np.float64)
    i = k
    # LT[i, k] = L[k, i] = lam^(k-i) for i < k   (lhsT of the triangular scan)
    LT = np.where(i[:, None] < k[None, :], LAM ** (k[None, :] - i[:, None]), 0.0)
    powv = (LAM ** k)[None, :]                      # [1, 128]
    vw = (LAM ** (P - i))[:, None]                  # [128, 1]
    f16 = np.float16
    return {
        "lt": LT.astype(f16),
        "powv": powv.astype(f16),
        "vw": vw.astype(f16),
        "mask": np.eye(P, dtype=f16),
    }


def _build_nc():
    nc = bacc.Bacc("TRN2", target_bir_lowering=False, debug=False,
                   num_devices=NCORES)
    x_d = nc.declare_dram_parameter("x", [P, NT, P], F16, isOutput=False)        # [i, t, e]
    xt_d = nc.declare_dram_parameter("xt", [P, CHUNK], F16, isOutput=False)      # [d, p]
    halo_d = nc.declare_dram_parameter("halo", [P, P], F16, isOutput=False)      # [i, e]
    w2_d = nc.declare_dram_parameter("w2", [P, P * P], F16, isOutput=False)      # [d, (e,f)]
    lt_d = nc.declare_dram_parameter("lt", [P, P], F16, isOutput=False)
    pow_d = nc.declare_dram_parameter("powv", [1, P], F16, isOutput=False)
    vw_d = nc.declare_dram_parameter("vw", [P, 1], F16, isOutput=False)
    mask_d = nc.declare_dram_parameter("mask", [P, P], F16, isOutput=False)
    out_d = nc.declare_dram_parameter("out", [P, NT, P], F32, isOutput=True)  # [p, t, f]

    mult = mybir.AluOpType.mult
    add = mybir.AluOpType.add

    with tile.TileContext(nc) as tc:
        with tc.tile_pool(name="consts", bufs=1) as consts:
            w2_sb = [consts.tile([P, 2048], F16, name=f"w2_sb{i}")
                     for i in range(8)]
            xt_sb = consts.tile([P, CHUNK], F16)
            x_sb = consts.tile([P, NT, P], F16)
            halo_sb = consts.tile([P, P], F16)
            lt_sb = consts.tile([P, P], F16)
            pow_sb = consts.tile([1, P], F16)
            vw_sb = consts.tile([P, 1], F16)
            mask_sb = consts.tile([P, P], F16)
            c_all = consts.tile([1, NT * P], F16)    # [1, (t,e)] carries
            s_sb = consts.tile([P, NT, P], F32)      # [p, t, e]
            acc = consts.tile([P, NT, P], F32)       # [p, t, f]
            y_sb = consts.tile([P, 2 * NGRP, 8, P], F16)       # [p, gi, e, f]
            y2_sb = consts.tile([P, 2 * NGRP, 4, P], F16)      # GP prefold out

            # critical-path-first DMA order: tiny weights + halo (carries),
            # then x halves, then xt; W2 streams on the scalar DGE queue in
            # consumption order so it never blocks these.
            nc.sync.dma_start(out=vw_sb[:, :], in_=vw_d[:, :])
            nc.sync.dma_start(out=halo_sb[:, :], in_=halo_d[:, :])
            nc.sync.dma_start(out=lt_sb[:, :], in_=lt_d[:, :])
            nc.sync.dma_start(out=pow_sb[:, :], in_=pow_d[:, :])
            nc.sync.dma_start(out=mask_sb[:, :], in_=mask_d[:, :])
            nc.sync.dma_start(out=x_sb[:, 0:4, :], in_=x_d[:, 0:4, :])
            nc.sync.dma_start(out=x_sb[:, 4:8, :], in_=x_d[:, 4:8, :])
            nc.sync.dma_start(out=xt_sb[:, :], in_=xt_d[:, :])
            for c in range(8):
                nc.scalar.dma_start(
                    out=w2_sb[c][:, :],
                    in_=w2_d[:, ds(2048 * c, 2048)])

            # ---- carries: c_t = s(tile_start t); c_t = v_{t-1} exactly ----
            with tc.tile_pool(name="psum_c", bufs=1, space="PSUM") as psum_c:
                c0_ps = psum_c.tile([1, P], F32)
                nc.tensor.matmul(c0_ps[:, :], lhsT=vw_sb[:, :],
                                 rhs=halo_sb[:, :], start=True, stop=True)
                vps_a = psum_c.tile([1, 4 * P], F32, tag="vps_a")
                vps_b = psum_c.tile([1, 4 * P], F32, tag="vps_b")
                nc.tensor.matmul(vps_a[:, :], lhsT=vw_sb[:, :],
                                 rhs=x_sb[:, 0:4, :], start=True, stop=True)
                nc.tensor.matmul(vps_b[:, :], lhsT=vw_sb[:, :],
                                 rhs=x_sb[:, 4:8, :], start=True, stop=True)
                nc.vector.tensor_copy(c_all[:, 0:P], c0_ps[:, :])
                nc.vector.tensor_copy(c_all[:, P:5 * P], vps_a[:, :])
                nc.vector.tensor_copy(c_all[:, 5 * P:8 * P], vps_b[:, 0:3 * P])

            # ---- main loop (s tiles interleaved two ahead) ----
            # psum_y takes 6 banks; one bank holds the two s slots and one
            # the two dacc slots.  Within each bank, accumulation groups are
            # serialized in program order (strict-FIFO fold drain).
            with tc.tile_pool(name="psum_s", bufs=1, space="PSUM") as psum_s, \
                 tc.tile_pool(name="psum_d", bufs=1, space="PSUM") as psum_d, \
                 tc.tile_pool(name="psum_y", bufs=3, space="PSUM") as psum_y:
                sslots = psum_s.tile([P, 2, P], F32)
                dslots = psum_d.tile([P, 2, P], F32)

                def emit_s(t):
                    sp = sslots[:, t % 2, :]
                    nc.tensor.matmul(sp, lhsT=lt_sb[:, :],
                                     rhs=x_sb[:, t, :], start=True, stop=False)
                    nc.tensor.matmul(sp, lhsT=pow_sb[:, :],
                                     rhs=c_all[:, ts(t, P)], start=False, stop=True)
                    nc.scalar.copy(s_sb[:, t, :], sp)

                emit_s(0)
                emit_s(1)

                DELAYS = {"T": 3, "S": 5}   # groups between evac and id-MMs
                fold_state = {}             # t -> [d2, dfold, n_emitted]
                pending = []                # (gi, due_gabs, t), FIFO

                def emit_id_mms(gq, tq):
                    # Folds 4 e-slices per matmul into the same 128
                    # accumulator columns: the stride-0 out dim revisits each
                    # PSUM element 4x (128 columns apart), and has_written
                    # semantics turn the revisits into accumulation.
                    st = fold_state[tq]
                    if (gq % NGRP) in PREFOLD_GP:
                        nc.tensor.matmul(
                            st[1], lhsT=mask_sb[:, :],
                            rhs=y2_sb[:, gq, :, :],
                            start=(st[2] == 0),
                            stop=(st[2] == N_DMM - 1))
                        st[2] += 1
                    else:
                        for h in range(2):
                            nc.tensor.matmul(
                                st[1], lhsT=mask_sb[:, :],
                                rhs=y_sb[:, gq, ds(4 * h, 4), :],
                                start=(st[2] == 0),
                                stop=(st[2] == N_DMM - 1))
                            st[2] += 1

                def emit_merge(tq):
                    st = fold_state.pop(tq)
                    assert st[2] == N_DMM
                    nc.vector.tensor_copy(acc[:, tq, :], st[0])
                    nc.sync.dma_start(out=out_d[:, tq, :], in_=acc[:, tq, :])

                for t in range(NT):
                    xt_t = xt_sb[:, ts(t, P)]
                    d2 = dslots[:, t % 2, :]
                    dfold = bass.AP(d2.tensor, d2.offset,
                                    [d2.ap[0], [0, 4], d2.ap[1]])
                    fold_state[t] = [d2, dfold, 0]

                    for g in range(NGRP):
                        gabs = t * NGRP + g
                        while pending and gabs >= pending[0][1]:
                            gq, _, tq = pending.pop(0)
                            emit_id_mms(gq, tq)
                        if g == 7 and t >= 1:
                            emit_merge(t - 1)
                        if g == 8 and t + 2 < NT:
                            emit_s(t + 2)
                        yp = psum_y.tile([P, 8, P], F32)
                        for h in range(2):
                            c = 2 * g + h
                            nc.tensor.matmul(
                                yp[:, ds(4 * h, 4), :], lhsT=xt_t,
                                rhs=w2_sb[c // 4][:, ds(512 * (c % 4), 512)],
                                start=True, stop=True)
                        r = ROUTE[g]
                        gi = (t % 2) * NGRP + g
                        if r == "T":
                            s3 = s_sb[:, t, ds(8 * g, 8)]
                            s3b = bass.AP(s3.tensor, s3.offset,
                                          s3.ap + [[0, P]])
                            nc.vector.tensor_tensor(
                                out=y_sb[:, gi, :, :], in0=yp[:, :, :],
                                in1=s3b, op=mult)
                        else:   # 'S': per-e scaled copy on ACT
                            for jj in range(8):
                                e = 8 * g + jj
                                nc.scalar.mul(
                                    out=y_sb[:, gi, jj, :],
                                    in_=yp[:, jj, :],
                                    mul=s_sb[:, t, e:e + 1])
                        delay = DELAYS[r]
                        if g in PREFOLD_GP:
                            nc.gpsimd.tensor_tensor(
                                out=y2_sb[:, gi, :, :],
                                in0=y_sb[:, gi, ds(0, 4), :],
                                in1=y_sb[:, gi, ds(4, 4), :],
                                op=add)
                            delay += 2
                        pending.append((gi, gabs + delay, t))
                while pending:
                    gq, _, tq = pending.pop(0)
                    emit_id_mms(gq, tq)
                emit_merge(NT - 1)
    nc.finalize()
    return nc


def _get_nc():
    if "nc" not in _CACHE:
        _CACHE["nc"] = _build_nc()
    return _CACHE["nc"]


def kernel(x, concept_map, _trace=False):
    global LAST_RESULTS
    x = np.asarray(x, dtype=np.float32)
    cm = np.asarray(concept_map, dtype=np.float32)
    assert x.shape == (B, S, D) and cm.shape == (D, D, D)

    consts = _host_constants()
    # W2[d, e*128+f] = cm[f, d, e]
    w2 = np.ascontiguousarray(
        np.transpose(cm, (1, 2, 0)).reshape(D, D * D).astype(np.float16))

    in_maps = []
    for core in range(NCORES):
        b, half = divmod(core, 2)
        lo = half * CHUNK
        xc = x[b, lo:lo + CHUNK]                          # [1024, 128]
        # [i, t, e] interleaved layout (partition = within-tile position)
        x_il = np.ascontiguousarray(
            xc.reshape(NT, P, D).transpose(1, 0, 2).astype(np.float16))
        xt = np.ascontiguousarray(xc.T.astype(np.float16))  # [d, p]
        if half == 0:
            halo = np.zeros((P, D), dtype=np.float16)
        else:
            halo = np.ascontiguousarray(
                x[b, lo - HALO:lo].astype(np.float16))    # [128, 128]
        in_maps.append({
            "x": x_il, "xt": xt, "halo": halo, "w2": w2, **consts,
        })

    nc = _get_nc()
    res = run_bass_kernel_spmd(nc, in_maps, list(range(NCORES)), trace=_trace)
    LAST_RESULTS = res

    out = np.empty((B, S, D), dtype=np.float32)
    for core in range(NCORES):
        b, half = divmod(core, 2)
        o = res.results[core]["out"]                      # [p, t, f]
        out[b, half * CHUNK:(half + 1) * CHUNK] = (
            o.transpose(1, 0, 2).reshape(CHUNK, D))
    return out
